# revision 37
# baseline (speedup 1.0000x reference)
"""Trainium2 Bass kernel for nn_MultiHeadAttention_7413113553038.

Sharding: 8 cores = (batch b in {0,1}) x (query block of 512). Each core
computes all 4 heads of attention for its 512 queries against the full 2048
keys of its batch, plus the output projection, residual add and LayerNorm for
its rows. No collectives needed.

Per-core strategy:
  - Host passes X_Q^T (block), X_K^T, X_V^T (pre-transposed + rolled), weights
    in fp16, and precomputed multiplicative Gaussian-bias band tables
    E = exp(bias) (neutral value 1 outside the causal band).
  - Q^T/K^T computed in [d, seq] layout directly (lhsT = W, rhs = X^T).
  - scores computed transposed: sT[k, q] = K Q^T (contraction d=64, heads at
    partition bases 0/64 so head pairs use distinct PE row groups).
  - p = exp(sT) with NO max-subtraction (scores are O(6), exp safe in f32),
    then p *= E_slice on the 6 k-chunk slots covering the causal band
    (X_K/X_V are rolled by q0-256 so the band sits on static slots 0..5).
  - V is augmented with a ones-column so ctxT = V_aug.T @ p yields the
    softmax denominator Z as psum row 64 for free.
  - 1/Z: Z rows are scattered to 128 partitions with tiny PE transposes,
    reciprocal'd at [128,16], transposed back and broadcast across partitions
    with selector matmuls; ctxT is scaled before the fc projection.
  - ctxT [dm, q] is exactly the lhsT layout the fc matmul needs; LayerNorm
    via bn_stats/bn_aggr; fc psum + residual + LN fused per 128-row chunk.
"""

import numpy as np

N_HEADS = 4
D_K = 64
B = 2
S = 2048
F = 256
QB = 512  # queries per core
P = 128
KC = S // P  # 16 k-chunks
SIGMA_HS = (5.0, 10.0, 20.0, 40.0)
LN_EPS = 1e-5
N_CORES = 8
# per-head causal-bias band width (g >= ~1e-4): ceil(4.292 * sigma)
BAND = (22, 43, 86, 172)
E01_W = 192
E25_W = 304


_CACHE = {}


def _gauss_tables():
    """Compact multiplicative Gaussian-bias band tables E = exp(g) in fp16,
    transposed-score layout (delta = q - k = off_t + j - i, off_t = 256-128t).

    Only the diagonal band where g >= ~1e-4 matters (fp16 rounds exp(g) to
    1.0 below ~5e-4 anyway), so the tables store just the band:
      e01 [4,128,192]: e01[h,i,m] = exp(g_h(m - i + 128)), slots 0,1
                       (slice col = (128 - 128t) + j)
      e25 [4,128,304]: e25[h,i,m] = exp(g_h(m - i)), slots 2..5
                       (slice col = j - 128*(t-2))
    g_h(d) = exp(-d^2 / (2 sigma_h^2)) for d >= 0 else 0.
    """
    i = np.arange(P, dtype=np.float64)[None, :, None]
    sig = np.asarray(SIGMA_HS, dtype=np.float64)[:, None, None]

    m01 = np.arange(E01_W, dtype=np.float64)[None, None, :]
    d01 = m01 - i + 128.0
    g01 = np.where(d01 >= 0, np.exp(-(d01 ** 2) / (2 * sig ** 2)), 0.0)

    m25 = np.arange(E25_W, dtype=np.float64)[None, None, :]
    d25 = m25 - i
    g25 = np.where(d25 >= 0, np.exp(-(d25 ** 2) / (2 * sig ** 2)), 0.0)
    return (
        np.exp(g01).astype(np.float16),
        np.exp(g25).astype(np.float16),
    )


def _build_program():
    import concourse.bass as bass  # noqa: F401
    import concourse.tile as tile
    from concourse import bacc, mybir
    from concourse.masks import make_identity

    f32 = mybir.dt.float32
    f16 = mybir.dt.float16
    AF = mybir.ActivationFunctionType
    ALU = mybir.AluOpType

    nc = bacc.Bacc("TRN2", target_bir_lowering=False, debug=False)

    # all inputs are pre-packed on the host into their exact SBUF layouts so
    # every DMA is a single fully-linear transfer
    xqt = nc.dram_tensor("xqt", [P, 2, QB], f16, kind="ExternalInput").ap()
    res = nc.dram_tensor("res", [P, 4, F], f16, kind="ExternalInput").ap()
    xkt = nc.dram_tensor("xkt", [4, P, 2, 512], f16, kind="ExternalInput").ap()
    xvt = nc.dram_tensor("xvt", [4, P, 2, 512], f16, kind="ExternalInput").ap()
    wq = nc.dram_tensor("wq8", [P, 2, F], f16, kind="ExternalInput").ap()
    wk = nc.dram_tensor("wk", [P, 2, F], f16, kind="ExternalInput").ap()
    wv = nc.dram_tensor("wv", [P, 2, F], f16, kind="ExternalInput").ap()
    wfc = nc.dram_tensor("wfc", [P, 2, F], f16, kind="ExternalInput").ap()
    e01 = nc.dram_tensor("e01", [P, N_HEADS, E01_W], f16, kind="ExternalInput").ap()
    e25 = nc.dram_tensor("e25", [P, N_HEADS, E25_W], f16, kind="ExternalInput").ap()
    out = nc.dram_tensor("out", [P, 4, F], f32, kind="ExternalOutput").ap()

    with tile.TileContext(nc) as tc:
        with (
            tc.tile_pool(name="wpool", bufs=1) as wpool,
            tc.tile_pool(name="xpool", bufs=1) as xpool,
            tc.tile_pool(name="proj", bufs=1) as proj,
            tc.tile_pool(name="mmps", bufs=2, space="PSUM") as mmps,
            tc.tile_pool(name="spsum", bufs=2, space="PSUM") as spsum,
            tc.tile_pool(name="cpsum", bufs=2, space="PSUM") as cpsum,
            tc.tile_pool(name="ptpool", bufs=3) as ptpool,
            tc.tile_pool(name="opool", bufs=4) as opool,
        ):
            # ---- load inputs: two HWDGE queues in parallel, ordered to
            # match the consumption order of the interleaved proj/attention
            # stream ----
            xqt_sb = xpool.tile([P, 2, QB], f16, tag="xqt")
            nc.sync.dma_start(xqt_sb, xqt)
            wq_sb = wpool.tile([P, 2, F], f16, tag="wq")
            nc.sync.dma_start(wq_sb, wq)
            wk_sb = wpool.tile([P, 2, F], f16, tag="wk")
            nc.sync.dma_start(wk_sb, wk)
            wv_sb = wpool.tile([P, 2, F], f16, tag="wv")
            nc.scalar.dma_start(wv_sb, wv)

            xkt_b = []
            xvt_b = []
            for nb in range(4):
                kb = xpool.tile([P, 2, 512], f16, tag=f"xkt{nb}", name=f"xkt{nb}")
                nc.sync.dma_start(kb, xkt[nb])
                xkt_b.append(kb)
                vb = xpool.tile([P, 2, 512], f16, tag=f"xvt{nb}", name=f"xvt{nb}")
                nc.scalar.dma_start(vb, xvt[nb])
                xvt_b.append(vb)
                if nb == 0:
                    e01_sb = wpool.tile([P, N_HEADS, E01_W], f16, tag="e01")
                    nc.sync.dma_start(e01_sb, e01)
                elif nb == 1:
                    e25_sb = wpool.tile([P, N_HEADS, E25_W], f16, tag="e25")
                    nc.sync.dma_start(e25_sb, e25)

            wfc_sb = wpool.tile([P, 2, F], f16, tag="wfc")
            nc.scalar.dma_start(wfc_sb, wfc)
            res_t = wpool.tile([P, 4, F], f16, tag="res")
            nc.scalar.dma_start(res_t, res)

            ident_f = wpool.tile([P, P], f32, tag="identf")
            make_identity(nc, ident_f)
            ones_t = wpool.tile([P, P], f32, tag="ones")
            nc.vector.memset(ones_t, 1.0)
            sel = wpool.tile([4, N_HEADS, P], f16, tag="sel")
            for h in range(N_HEADS):
                nc.vector.tensor_scalar_mul(
                    sel[0:4, h, :], ones_t[0:4, :], ident_f[0:4, h:h + 1]
                )
            eps_t = wpool.tile([P, 1], f32, tag="eps")
            nc.vector.memset(eps_t, LN_EPS)

            # ---- persistent tiles ----
            qt_sb = proj.tile([P, 2, QB], f16, tag="qt")
            kt_b = [
                proj.tile([P, 2, 512], f16, tag=f"kt{nb}", name=f"kt{nb}")
                for nb in range(4)
            ]
            v_b = [
                proj.tile([P, 4, N_HEADS, 65], f16, tag=f"v{nb}", name=f"v{nb}")
                for nb in range(4)
            ]
            ctx_sb = proj.tile([P, 2, QB], f16, tag="ctx")
            ztmp_z = proj.tile([P, N_HEADS, QB], f32, tag="z")
            fcacc = proj.tile([P, 4, F], f32, tag="fcacc")
            o_sb = proj.tile([P, 4, F], f32, tag="osb")

            # residual pre-folded into the fc accumulator
            nc.vector.tensor_copy(fcacc, res_t)


            # ---- QT projection ----
            for g in range(2):
                ps = mmps.tile([P, 512], f32, tag="mm")
                for c in range(2):
                    nc.tensor.matmul(
                        ps,
                        wq_sb[:, c, g * P:(g + 1) * P],
                        xqt_sb[:, c, :],
                        start=(c == 0),
                        stop=(c == 1),
                    )
                nc.vector.tensor_copy(qt_sb[:, g, :], ps)

            def project_kt(nb):
                for g in range(2):
                    ps = mmps.tile([P, 512], f32, tag="mm", name=f"psk{nb}{g}")
                    for c in range(2):
                        nc.tensor.matmul(
                            ps,
                            wk_sb[:, c, g * P:(g + 1) * P],
                            xkt_b[nb][:, c, :],
                            start=(c == 0),
                            stop=(c == 1),
                        )
                    nc.vector.tensor_copy(kt_b[nb][:, g, :], ps)

            def project_v(nb):
                for j in range(4):
                    ps = mmps.tile([P, 512], f32, tag="mm", name=f"psv{nb}{j}")
                    psv = ps[:, :F]
                    for c in range(2):
                        nc.tensor.matmul(
                            psv,
                            xvt_b[nb][:, c, j * P:(j + 1) * P],
                            wv_sb[:, c, :],
                            start=(c == 0),
                            stop=(c == 1),
                        )
                    nc.vector.tensor_copy(
                        v_b[nb][:, j, :, 0:64],
                        psv.rearrange("p (h d) -> p h d", h=N_HEADS),
                    )
                nc.vector.tensor_copy(
                    v_b[nb][:, :, :, 64:65],
                    ones_t[:, 0:4 * N_HEADS].rearrange(
                        "p (j h one) -> p j h one", j=4, h=N_HEADS, one=1
                    ),
                )

            def attn_sc(G, kc):
                """Scores + exp for one k-chunk of head pair G; returns pt."""
                ps = spsum.tile([P, 2 * QB], f32, tag="sc", name=f"sc{G[0]}_{kc}")
                for hi, h in enumerate(G):
                    g, po = h // 2, (h % 2) * 64
                    nc.tensor.matmul(
                        ps[:, hi * QB:(hi + 1) * QB],
                        kt_b[kc // 4][po:po + 64, g, (kc % 4) * P:(kc % 4 + 1) * P],
                        qt_sb[po:po + 64, g, :],
                        start=True,
                        stop=True,
                    )
                pt = ptpool.tile([P, 2 * QB], f16, tag="pt", name=f"pt{G[0]}_{kc}")
                nc.scalar.activation(pt, ps, AF.Exp)
                return pt

            def attn_pv(G, ctxps, kc, pt):
                """Band multiply + PV accumulate for one k-chunk."""
                for hi, h in enumerate(G):
                    if kc <= 5:
                        off_t = 256 - 128 * kc
                        j0 = max(0, -off_t)
                        j1 = min(512, BAND[h] + 128 - off_t)
                        j1 = min(512, (j1 + 7) & ~7)
                        if j1 > j0:
                            if kc <= 1:
                                c0 = (128 - 128 * kc) + j0
                                esl = e01_sb[:, h, c0:c0 + (j1 - j0)]
                            else:
                                c0 = j0 - 128 * (kc - 2)
                                esl = e25_sb[:, h, c0:c0 + (j1 - j0)]
                            nc.vector.tensor_mul(
                                pt[:, hi * QB + j0:hi * QB + j1],
                                pt[:, hi * QB + j0:hi * QB + j1],
                                esl,
                            )
                    nc.tensor.matmul(
                        ctxps[hi][0:65, :],
                        v_b[kc // 4][:, kc % 4, h, 0:65],
                        pt[:, hi * QB:(hi + 1) * QB],
                        start=(kc == 0),
                        stop=(kc == KC - 1),
                    )

            def attn_kc(G, ctxps, kc):
                attn_pv(G, ctxps, kc, attn_sc(G, kc))

            def epilogue_steps(G, ctxps):
                """Per-group epilogue as a list of emission closures, so group
                0's steps can be drip-fed into group 1's instruction stream
                (avoids head-of-line blocking on the strict engine queues)."""
                gg = G[0] // 2
                state = {}

                def s_copies():
                    for hi, h in enumerate(G):
                        po = (h % 2) * 64
                        nc.vector.tensor_copy(
                            ctx_sb[po:po + 64, gg, :], ctxps[hi][0:64, :]
                        )
                        if gg == 1:
                            # tail-latency critical: run z extraction on the
                            # (otherwise idle) scalar engine in parallel
                            nc.scalar.copy(
                                ztmp_z[64:65, h, :], ctxps[hi][64:65, :]
                            )
                        else:
                            nc.vector.tensor_copy(
                                ztmp_z[64:65, h, :], ctxps[hi][64:65, :]
                            )

                def s_fwd_t():
                    zt_g = mmps.tile([P, 512], f32, tag="mm", name=f"zt{gg}")
                    state["zt_g"] = zt_g
                    for hi, h in enumerate(G):
                        for qc in range(4):
                            nc.tensor.transpose(
                                zt_g[:, hi * 4 + qc:hi * 4 + qc + 1],
                                ztmp_z[64:65, h, qc * P:(qc + 1) * P],
                                ident_f[64:65, 64:65],
                            )
                    ztc = opool.tile([P, 8], f32, tag="ztc", name=f"ztc{gg}")
                    state["ztc"] = ztc
                    nc.vector.tensor_copy(ztc, zt_g[:, 0:8])
                    nc.vector.reciprocal(ztc, ztc)

                def s_back_t():
                    ztc = state["ztc"]
                    rz_ps = mmps.tile([P, 512], f32, tag="mm", name=f"rz{gg}")
                    for hi in range(2):
                        nc.tensor.transpose(
                            rz_ps[0:4, hi * P:(hi + 1) * P],
                            ztc[:, hi * 4:(hi + 1) * 4],
                            ident_f,
                        )
                    rz4 = opool.tile([4, 2, P], f16, tag="rz4", name=f"rz4{gg}")
                    state["rz4"] = rz4
                    nc.vector.tensor_copy(
                        rz4, rz_ps[0:4, 0:2 * P].rearrange("p (h j) -> p h j", h=2)
                    )

                def s_zb(hi):
                    def emit():
                        h = G[hi]
                        po = (h % 2) * 64
                        zb = mmps.tile([P, 512], f32, tag="mm", name=f"zb{h}")
                        for qc in range(4):
                            nc.tensor.matmul(
                                zb[:, qc * P:(qc + 1) * P],
                                sel[0:4, qc, :],
                                state["rz4"][0:4, hi, :],
                                start=True,
                                stop=True,
                            )
                        nc.vector.tensor_mul(
                            ctx_sb[po:po + 64, gg, :],
                            ctx_sb[po:po + 64, gg, :],
                            zb[po:po + 64, :],
                        )
                    return emit

                def s_fc(qc):
                    def emit():
                        pso = mmps.tile(
                            [P, 512], f32, tag="mm", name=f"pso{gg}{qc}"
                        )
                        pso = pso[:, :F]
                        nc.tensor.matmul(
                            pso,
                            ctx_sb[:, gg, qc * P:(qc + 1) * P],
                            wfc_sb[:, gg, :],
                            start=True,
                            stop=True,
                        )
                        if gg == 0:
                            nc.vector.tensor_add(
                                fcacc[:, qc, :], fcacc[:, qc, :], pso
                            )
                        else:
                            x_t = opool.tile([P, F], f32, tag="x", name=f"x{qc}")
                            nc.vector.tensor_add(x_t, pso, fcacc[:, qc, :])
                            st = opool.tile([P, 6], f32, tag="st", name=f"st{qc}")
                            nc.vector.bn_stats(st, x_t)
                            mv = opool.tile([P, 2], f32, tag="mv", name=f"mv{qc}")
                            nc.vector.bn_aggr(mv, st)
                            nc.scalar.activation(
                                mv[:, 1:2], mv[:, 1:2], AF.Sqrt,
                                bias=eps_t, scale=1.0,
                            )
                            nc.vector.reciprocal(mv[:, 1:2], mv[:, 1:2])
                            nc.vector.tensor_scalar(
                                o_sb[:, qc, :],
                                x_t,
                                mv[:, 0:1],
                                mv[:, 1:2],
                                op0=ALU.subtract,
                                op1=ALU.mult,
                            )
                            if qc == 3:
                                nc.sync.dma_start(out, o_sb)
                    return emit

                return [s_copies, s_fwd_t, s_back_t, s_zb(0), s_zb(1),
                        s_fc(0), s_fc(1), s_fc(2), s_fc(3)]

            # ---- group 0: projections interleaved with its attention ----
            G0, G1 = (0, 1), (2, 3)
            ctxps0 = [
                cpsum.tile([P, QB], f32, tag="ctxp", name=f"ctxp{hh}")
                for hh in G0
            ]
            project_kt(0)
            project_v(0)
            for nb in range(4):
                for i, kc in enumerate(range(4 * nb, 4 * nb + 4)):
                    attn_kc(G0, ctxps0, kc)
                    if nb < 3:
                        if i == 2:
                            project_kt(nb + 1)
                        elif i == 3:
                            project_v(nb + 1)
            steps0 = epilogue_steps(G0, ctxps0)
            steps0[0]()  # ctx/z copies (DVE only, releases ctxps slots)

            # ---- group 1: attention with group-0 epilogue drip-fed in ----
            ctxps1 = [
                cpsum.tile([P, QB], f32, tag="ctxp", name=f"ctxp{hh}")
                for hh in G1
            ]
            drip = {3: steps0[1], 5: steps0[2], 7: steps0[3], 8: steps0[4],
                    10: steps0[5], 11: steps0[6], 12: steps0[7], 13: steps0[8]}
            for kc in range(KC):
                attn_kc(G1, ctxps1, kc)
                if kc in drip:
                    drip[kc]()
            for step in epilogue_steps(G1, ctxps1):
                step()

    nc.compile()
    return nc


def get_nc():
    if "nc" not in _CACHE:
        _CACHE["nc"] = _build_program()
    return _CACHE["nc"]


def make_in_maps(input_Q, input_K, input_V, W_Q, W_K, W_V, W_fc):
    c16 = lambda a: np.ascontiguousarray(
        np.asarray(a, dtype=np.float32), dtype=np.float16
    )
    # pack [in, out]-style matrices to SBUF layout [p, c, out]
    pk_w = lambda w: c16(np.asarray(w, np.float32).reshape(2, P, -1).transpose(1, 0, 2))
    # pack an activation block X [seq, F] to X^T SBUF layout [p, c, seq]
    pk_t = lambda x: c16(np.asarray(x, np.float32).T.reshape(2, P, -1).transpose(1, 0, 2))
    # pack a rolled key/value matrix [2048, F] to per-block X^T [nb, p, c, 512]
    pk_x = lambda x: c16(
        np.asarray(x, np.float32).reshape(4, 512, 2, P).transpose(0, 3, 2, 1)
    )
    e01t, e25t = _gauss_tables()
    e01 = np.ascontiguousarray(e01t.transpose(1, 0, 2))
    e25 = np.ascontiguousarray(e25t.transpose(1, 0, 2))
    e01_neutral = np.ones_like(e01)
    wq8 = pk_w(np.asarray(W_Q, np.float32) / np.float32(np.sqrt(D_K)))
    wk = pk_w(W_K)
    wv = pk_w(W_V)
    wfc = pk_w(W_fc)
    in_maps = []
    for c in range(N_CORES):
        b, qb = divmod(c, 4)
        q0 = qb * QB
        r = (q0 - 256) % S
        xq_blk = np.asarray(input_Q[b][q0:q0 + QB], np.float32)
        xk_rot = np.roll(np.asarray(input_K[b], np.float32), -r, axis=0)
        xv_rot = np.roll(np.asarray(input_V[b], np.float32), -r, axis=0)
        in_maps.append({
            "xqt": pk_t(xq_blk),
            "res": c16(xq_blk.reshape(4, P, F).transpose(1, 0, 2)),
            "xkt": pk_x(xk_rot),
            "xvt": pk_x(xv_rot),
            "wq8": wq8,
            "wk": wk,
            "wv": wv,
            "wfc": wfc,
            "e01": e01_neutral if q0 == 0 else e01,
            "e25": e25,
        })
    return in_maps


def kernel(input_Q, input_K, input_V, W_Q, W_K, W_V, W_fc, attn_mask=None):
    from concourse.bass_utils import run_bass_kernel_spmd

    nc = get_nc()
    in_maps = make_in_maps(input_Q, input_K, input_V, W_Q, W_K, W_V, W_fc)
    res = run_bass_kernel_spmd(nc, in_maps, core_ids=list(range(N_CORES)))
    out = np.empty((B, S, F), dtype=np.float32)
    for c in range(N_CORES):
        b, qb = divmod(c, 4)
        o = res.results[c]["out"]
        out[b, qb * QB:(qb + 1) * QB, :] = o.transpose(1, 0, 2).reshape(QB, F)
    return out
